# revision 1
# baseline (speedup 1.0000x reference)
"""Trainium2 Bass kernel for nn_DEC_LargeCNN2Int (turbo-decoder CNN).

Data-parallel over 8 NeuronCores (32 samples each). Per core, per sample:
12 stack-instances of [conv0(7->100,K5) + 4x conv(100->100,K5), ELU] +
linear(100->5), with token interleaving between stacks done on-chip via
GPSIMD ap_gather in an octet layout (16 samples x 8 rows per 128-partition
tile). Convs run as 5 accumulating tap-matmuls (fp32r) per layer over a
halo'd channel-major activation tile.
"""
import numpy as np

import concourse.bass as bass
import concourse.mybir as mybir
import concourse.tile as tile
from concourse import bacc
from concourse.bass_utils import run_bass_kernel_spmd

F32 = mybir.dt.float32
F32R = mybir.dt.float32r
I16 = mybir.dt.int16
AF = mybir.ActivationFunctionType
ALU = mybir.AluOpType

B, L, FT, NUM_ITER, NL, UNIT, K = 256, 2048, 5, 6, 5, 100, 5
N_CORES = 8
TAPS = [2, 0, 1, 3, 4]          # tap 2 first: full coverage -> start=True
NCH = L // 512                  # 4 psum chunks


def _wrap_idx(t, groups):
    """ap_gather index layout: idx j at [j%16, j//16], replicated per 16-row group."""
    w = np.zeros((16, L // 16), np.int16)
    w[np.arange(L) % 16, np.arange(L) // 16] = t.astype(np.int16)
    return np.tile(w, (groups, 1))


def build_host_inputs(inputs, spg, n_iter):
    """Host-side prep. Returns (shared dict, per-core list of dicts)."""
    n_inst = 2 * n_iter
    bpc = B // N_CORES                      # samples per core (full cfg: 32)
    n_groups = 2 if spg == 16 else 1        # small configs: single group
    use = n_groups * spg                    # samples actually computed per core

    p1 = np.asarray(inputs['p_array1']).astype(np.int64)
    p2 = np.asarray(inputs['p_array2']).astype(np.int64)
    inv1 = np.argsort(p1)
    inv2 = np.argsort(p2)
    t1 = inv2[p1]
    t2 = inv1[p2]

    received = np.asarray(inputs['received'], np.float32)
    r_sys, r_par1, r_par2 = received[:, :, 0], received[:, :, 1], received[:, :, 2]
    s1_sys = r_sys[:, p1]
    s1_par = r_par2[:, inv2[p1]]
    s2_sys = r_sys[:, p2]
    s2_par = r_par1[:, inv1[p2]]

    conv0_w = np.asarray(inputs['conv0_w'], np.float32)
    conv0_b = np.asarray(inputs['conv0_b'], np.float32)
    convs_w = np.asarray(inputs['convs_w'], np.float32)
    convs_b = np.asarray(inputs['convs_b'], np.float32)
    lin1_w = np.asarray(inputs['lin1_w'], np.float32)
    lin1_b = np.asarray(inputs['lin1_b'], np.float32)
    lin2_w = np.asarray(inputs['lin2_w'], np.float32)
    lin2_b = np.asarray(inputs['lin2_b'], np.float32)
    lin2_last_w = np.asarray(inputs['lin2_last_w'], np.float32)

    # masked conv0 lhsT: [inst, 128, 20*100]
    c0m = np.zeros((n_inst, 128, 20 * UNIT), np.float32)
    # mid-layer lhsT: [inst, 100, 20*100]
    cwm = np.zeros((n_inst, UNIT, 20 * UNIT), np.float32)
    cbias = np.zeros((UNIT, n_inst, NL, 2), np.float32)
    # fused linear lhsT: rows 0..99 = w^T, 100..104 = -I (extrinsic), 105 = bias
    linw = np.zeros((UNIT + FT + 1, n_inst, FT), np.float32)
    linlast = np.zeros((UNIT + FT + 1, 1), np.float32)
    linlast[:UNIT, 0] = lin2_last_w[0]

    for inst in range(n_inst):
        idx, col = divmod(inst, 2)
        w0 = conv0_w[idx, col]              # [100, 7, 5]
        for v in range(4):
            for k in range(K):
                blk = c0m[inst, :, (v * K + k) * UNIT:(v * K + k + 1) * UNIT]
                for j in range(4):
                    blk[32 * j + 8 * v:32 * j + 8 * v + 7, :] = w0[:, :, k].T
        for li in range(1, NL):
            wl = convs_w[idx, col, li - 1]  # [100out, 100in, 5]
            for k in range(K):
                cwm[inst, :, ((li - 1) * K + k) * UNIT:((li - 1) * K + k + 1) * UNIT] \
                    = wl[:, :, k].T
        for li in range(NL):
            b = conv0_b[idx, col] if li == 0 else convs_b[idx, col, li - 1]
            cbias[:, inst, li, 0] = -b
            cbias[:, inst, li, 1] = b - 1.0
        if inst == n_inst - 1:
            pass                            # last instance uses linlast
        else:
            lw = lin1_w[idx] if col == 0 else lin2_w[idx]
            lb = lin1_b[idx] if col == 0 else lin2_b[idx]
            linw[:UNIT, inst, :] = lw.T
            if inst > 0:
                linw[UNIT:UNIT + FT, inst, :] = -np.eye(FT, dtype=np.float32)
            linw[UNIT + FT, inst, :] = lb

    idx_t1 = _wrap_idx(t1, 8)
    idx_t2 = _wrap_idx(t2, 8)
    idx_o = _wrap_idx(inv2, 1)

    shared = {
        'c0m': c0m, 'cwm': cwm,
        'cbias': np.ascontiguousarray(cbias.reshape(UNIT, -1)),
        'linw': np.ascontiguousarray(linw.reshape(UNIT + FT + 1, -1)),
        'linlast': linlast,
        'idx_t1': idx_t1, 'idx_t2': idx_t2, 'idx_o': idx_o,
    }

    per_core = []
    for c in range(N_CORES):
        lo = c * bpc
        ta = np.zeros((n_groups, 128, L + 4), np.float32)
        s1i = np.zeros((n_groups, 128, L), np.float32)
        s2i = np.zeros((n_groups, 128, L), np.float32)
        for g in range(n_groups):
            for si in range(spg):
                s = lo + g * spg + si
                ta[g, 8 * si + 0, 2:L + 2] = s1_sys[s]
                ta[g, 8 * si + 1, 2:L + 2] = r_par1[s]
                s1i[g, 8 * si + 0] = s1_sys[s]
                s1i[g, 8 * si + 1] = s1_par[s]
                s2i[g, 8 * si + 0] = s2_sys[s]
                s2i[g, 8 * si + 1] = s2_par[s]
        m = dict(shared)
        m['ta_init'] = ta
        m['s1_init'] = s1i
        m['s2_init'] = s2i
        per_core.append(m)
    return shared, per_core, use


def build_program(spg=16, n_iter=NUM_ITER):
    """Emit the Bass/Tile program. spg = samples per group (<=16)."""
    import os
    KO = set(filter(None, os.environ.get("BASS_KNOCKOUT", "").split(",")))
    n_inst = 2 * n_iter
    n_groups = 2 if spg == 16 else 1
    ch = 8 * spg                       # used octet rows (128 at spg=16)
    chg = ((ch + 15) // 16) * 16       # gather channels (mult of 16)

    nc = bacc.Bacc('TRN2', target_bir_lowering=False, debug=False)

    ta_d = nc.dram_tensor("ta_init", [n_groups, 128, L + 4], F32R,
                          kind="ExternalInput")
    s1_d = nc.dram_tensor("s1_init", [n_groups, 128, L], F32, kind="ExternalInput")
    s2_d = nc.dram_tensor("s2_init", [n_groups, 128, L], F32, kind="ExternalInput")
    c0m_d = nc.dram_tensor("c0m", [n_inst, 128, 20 * UNIT], F32R,
                           kind="ExternalInput")
    cwm_d = nc.dram_tensor("cwm", [n_inst, UNIT, 20 * UNIT], F32R,
                           kind="ExternalInput")
    cb_d = nc.dram_tensor("cbias", [UNIT, n_inst * NL * 2], F32,
                          kind="ExternalInput")
    lw_d = nc.dram_tensor("linw", [UNIT + FT + 1, n_inst * FT], F32R,
                          kind="ExternalInput")
    ll_d = nc.dram_tensor("linlast", [UNIT + FT + 1, 1], F32R,
                          kind="ExternalInput")
    it1_d = nc.dram_tensor("idx_t1", [128, L // 16], I16, kind="ExternalInput")
    it2_d = nc.dram_tensor("idx_t2", [128, L // 16], I16, kind="ExternalInput")
    io_d = nc.dram_tensor("idx_o", [16, L // 16], I16, kind="ExternalInput")
    out_d = nc.dram_tensor("out", [n_groups * spg, L], F32, kind="ExternalOutput")

    with tile.TileContext(nc) as tc:
        with tc.tile_pool(name="persist", bufs=1) as pp, \
             tc.tile_pool(name="wts", bufs=2) as wp, \
             tc.tile_pool(name="elu", bufs=2) as ep, \
             tc.tile_pool(name="ps", bufs=2, space="PSUM") as ps:

            # ---- persistent tiles ----
            TA = [pp.tile([128, L + 4], F32R, tag=f"TA{g}", name=f"TA{g}") for g in range(n_groups)]
            TB = [pp.tile([128, L + 4], F32R, tag=f"TB{g}", name=f"TB{g}") for g in range(n_groups)]
            Tf = [pp.tile([128, L], F32, tag=f"Tf{g}", name=f"Tf{g}") for g in range(n_groups)]
            S1 = [pp.tile([128, L], F32, tag=f"S1{g}", name=f"S1{g}") for g in range(n_groups)]
            S2 = [pp.tile([128, L], F32, tag=f"S2{g}", name=f"S2{g}") for g in range(n_groups)]
            XB = [[pp.tile([106 if i == 1 else UNIT, L + 4], F32R,
                           tag=f"XB{p}_{i}", name=f"XB{p}_{i}") for i in range(3)]
                  for p in range(2)]
            cbias_t = pp.tile([UNIT, n_inst * NL * 2], F32, tag="cbias")
            linw_t = pp.tile([UNIT + FT + 1, n_inst * FT], F32R, tag="linw")
            linlast_t = pp.tile([UNIT + FT + 1, 1], F32R, tag="linlast")
            it1_t = pp.tile([128, L // 16], I16, tag="it1")
            it2_t = pp.tile([128, L // 16], I16, tag="it2")
            io_t = pp.tile([16, L // 16], I16, tag="io")

            # ---- init ----
            for g in range(n_groups):
                nc.vector.memset(TB[g][:, :].bitcast(F32), 0.0)
                nc.vector.memset(Tf[g][:, :], 0.0)
                nc.sync.dma_start(out=TA[g], in_=ta_d[g])
                nc.sync.dma_start(out=S1[g], in_=s1_d[g])
                nc.sync.dma_start(out=S2[g], in_=s2_d[g])
            for pset in XB:
                for xb in pset:
                    nc.vector.memset(xb[:, :].bitcast(F32), 0.0)
            # const-1 row 105 of XB[p][1] (rows 96..104 harmless: data rows get
            # overwritten by ELU; prior rows pair with zero lhsT rows at inst 0)
            for pset in XB:
                nc.vector.memset(pset[1][96:106, 2:L + 2].bitcast(F32), 1.0)
            nc.sync.dma_start(out=cbias_t, in_=cb_d[:, :])
            nc.sync.dma_start(out=linw_t, in_=lw_d[:, :])
            nc.sync.dma_start(out=linlast_t, in_=ll_d[:, :])
            nc.sync.dma_start(out=it1_t, in_=it1_d[:, :])
            nc.sync.dma_start(out=it2_t, in_=it2_d[:, :])
            nc.sync.dma_start(out=io_t, in_=io_d[:, :])

            def elu(psum, inst, li, xout, lo, hi):
                """xout[0:100, lo+2:hi+2] = elu(psum[:, lo:hi] + b)."""
                bcol = (inst * NL + li) * 2
                n = hi - lo
                ret = ep.tile([UNIT, L // 2], F32, tag="ret")
                if "elu_act" not in KO:
                    nc.scalar.activation(ret[:, 0:n], psum[:, lo:hi], AF.Relu,
                                         bias=cbias_t[:, bcol:bcol + 1], scale=-1.0)
                    nc.scalar.activation(ret[:, 0:n], ret[:, 0:n], AF.Exp,
                                         scale=-1.0)
                if "elu_dve" not in KO:
                    nc.vector.tensor_scalar(xout[0:UNIT, lo + 2:hi + 2],
                                            psum[:, lo:hi],
                                            cbias_t[:, bcol + 1:bcol + 2], -1.0,
                                            ALU.add, ALU.max)
                    nc.vector.tensor_tensor(xout[0:UNIT, lo + 2:hi + 2],
                                            xout[0:UNIT, lo + 2:hi + 2],
                                            ret[:, 0:n], ALU.add)

            for inst in range(n_inst):
                idx, col = divmod(inst, 2)
                last = inst == n_inst - 1
                c0t = wp.tile([128, 20 * UNIT], F32R, tag="c0t")
                cwt = wp.tile([UNIT, 20 * UNIT], F32R, tag="cwt")
                nc.sync.dma_start(out=c0t, in_=c0m_d[inst])
                nc.sync.dma_start(out=cwt, in_=cwm_d[inst])

                # gathers feeding this column's conv0 input
                for g in range(n_groups):
                    if "gather" in KO:
                        continue
                    if col == 0:
                        if idx == 0:
                            continue        # TA preloaded (prior = 0)
                        nc.gpsimd.ap_gather(Tf[g][0:chg, :], S2[g][0:chg, :],
                                            it1_t[0:chg, :], channels=chg,
                                            num_elems=L, d=1, num_idxs=L)
                        nc.scalar.copy(TA[g][:, 2:L + 2], Tf[g][:, :])
                    else:
                        nc.gpsimd.ap_gather(Tf[g][0:chg, :], S1[g][0:chg, :],
                                            it2_t[0:chg, :], channels=chg,
                                            num_elems=L, d=1, num_idxs=L)
                        nc.scalar.copy(TB[g][:, 2:L + 2], Tf[g][:, :])

                for g in range(n_groups):
                    T = TA[g] if col == 0 else TB[g]

                    def conv_layer(sp, si, li):
                        """One conv layer for sample si (pair slot sp)."""
                        j, v = divmod(si, 4)
                        pt = ps.tile([UNIT, L], F32, tag="ps", name="pt")
                        for c in range(NCH):
                            for t, k in (() if "conv" in KO
                                         else list(enumerate(TAPS))):
                                if li == 0:
                                    nc.tensor.matmul(
                                        pt[:, c * 512:(c + 1) * 512],
                                        c0t[32 * j:32 * j + 32,
                                            (v * K + k) * UNIT:
                                            (v * K + k + 1) * UNIT],
                                        T[32 * j:32 * j + 32,
                                          c * 512 + k:c * 512 + k + 512],
                                        start=(t == 0), stop=(t == 4),
                                        tile_position=(32 * j, 0))
                                else:
                                    xin = XB[sp][(li - 1) % 3]
                                    nc.tensor.matmul(
                                        pt[:, c * 512:(c + 1) * 512],
                                        cwt[:, ((li - 1) * K + k) * UNIT:
                                            ((li - 1) * K + k + 1) * UNIT],
                                        xin[0:UNIT,
                                            c * 512 + k:c * 512 + k + 512],
                                        start=(t == 0), stop=(t == 4))
                            if c == 1:
                                elu(pt, inst, li, XB[sp][li % 3], 0, L // 2)
                        elu(pt, inst, li, XB[sp][li % 3], L // 2, L)

                    def lin_stage(sp, si):
                        x5 = XB[sp][1]
                        rows = slice(8 * si + 2, 8 * si + 7)
                        if inst > 0:
                            nc.sync.dma_start(
                                out=x5[UNIT:UNIT + FT, 2:L + 2],
                                in_=Tf[g][rows, :].bitcast(F32R))
                        m = 1 if last else FT
                        wsl = linlast_t[:, 0:1] if last \
                            else linw_t[:, inst * FT:inst * FT + FT]
                        pslin = ps.tile([m, L], F32, tag="ps", name="pslin")
                        for c in range(NCH):
                            nc.tensor.matmul(
                                pslin[:, c * 512:(c + 1) * 512], wsl,
                                x5[:, c * 512 + 2:c * 512 + 514],
                                start=True, stop=True)
                        if last:
                            stg1 = ep.tile([1, L], F32, tag="stg1")
                            nc.scalar.activation(stg1, pslin[0:1, :], AF.Tanh,
                                                 scale=0.5)
                            nc.vector.tensor_scalar(stg1, stg1, 0.5, 0.5,
                                                    ALU.mult, ALU.add)
                            nc.sync.dma_start(out=Tf[g][si:si + 1, :],
                                              in_=stg1)
                        else:
                            Sd = S1[g] if col == 0 else S2[g]
                            stg = ep.tile([FT, L], F32, tag="stg")
                            nc.vector.tensor_copy(stg, pslin)
                            nc.sync.dma_start(out=Sd[rows, :], in_=stg)

                    for p in range(spg // 2):
                        sA, sB = 2 * p, 2 * p + 1
                        for li in range(NL):
                            conv_layer(0, sA, li)
                            conv_layer(1, sB, li)
                        if "lin" not in KO:
                            lin_stage(0, sA)
                            lin_stage(1, sB)

            # final: out[l] = sig[inv2[l]]
            for g in range(n_groups):
                nc.gpsimd.ap_gather(S1[g][0:16, :], Tf[g][0:16, :],
                                    io_t[0:16, :], channels=16,
                                    num_elems=L, d=1, num_idxs=L)
                nc.sync.dma_start(out=out_d[g * spg:g * spg + spg, :],
                                  in_=S1[g][0:spg, :])

    nc.compile()
    return nc


_PROG_CACHE = {}


def _get_prog(spg, n_iter):
    key = (spg, n_iter)
    if key not in _PROG_CACHE:
        _PROG_CACHE[key] = build_program(spg, n_iter)
    return _PROG_CACHE[key]


def run(inputs, spg=16, n_iter=NUM_ITER, cores=N_CORES, **spmd_kwargs):
    nc = _get_prog(spg, n_iter)
    _, per_core, use = build_host_inputs(inputs, spg, n_iter)
    res = run_bass_kernel_spmd(nc, per_core[:cores],
                               core_ids=list(range(cores)), **spmd_kwargs)
    return res, use


def kernel(**inputs):
    res, use = run(inputs)
    bpc = B // N_CORES
    out = np.empty((B, L, 1), np.float32)
    for c in range(N_CORES):
        out[c * bpc:(c + 1) * bpc, :, 0] = res.results[c]["out"]
    return out



# revision 15
# speedup vs baseline: 7.5903x; 7.5903x over previous
"""Trainium2 Bass kernel for nn_DEC_LargeCNN2Int (turbo-decoder CNN).

Data-parallel over 8 NeuronCores (32 samples each). Per core, per sample:
12 stack-instances of [conv0(7->100,K5) + 4x conv(100->100,K5), ELU] +
linear(100->5), with token interleaving between stacks done on-chip via
GPSIMD ap_gather in an octet layout (16 samples x 8 rows per 128-partition
tile). Convs run as 5 accumulating tap-matmuls (fp32r) per layer over a
halo'd channel-major activation tile.

Activations are stored in an offset representation x' = x + 1 so that the
conv bias (adjusted to b' = b - sum(W)) can ride a constant-1 contraction
row inside the matmul: PSUM then holds z + 1 directly and
ELU(z) + 1 = max(min(exp(z), 1), z + 1) takes just one scalar-engine Exp
and one fused scalar_tensor_tensor per chunk.
"""
import numpy as np

import concourse.bass as bass
import concourse.mybir as mybir
import concourse.tile as tile
from concourse import bacc
from concourse.bass_utils import run_bass_kernel_spmd

F32 = mybir.dt.float32
F32R = mybir.dt.float32r
I16 = mybir.dt.int16
AF = mybir.ActivationFunctionType
ALU = mybir.AluOpType

B, L, FT, NUM_ITER, NL, UNIT, K = 256, 2048, 5, 6, 5, 100, 5
N_CORES = 8
TAPS = [2, 0, 1, 3, 4]          # tap 2 first: full coverage -> start=True
NCH = L // 512                  # 4 psum chunks
CW_ROWS = UNIT + 1              # mid-layer lhsT rows (100 weights + bias row)


def _wrap_idx(t, groups):
    """ap_gather index layout: idx j at [j%16, j//16], replicated per 16-row group."""
    w = np.zeros((16, L // 16), np.int16)
    w[np.arange(L) % 16, np.arange(L) // 16] = t.astype(np.int16)
    return np.tile(w, (groups, 1))


def build_host_inputs(inputs, spg, n_iter):
    """Host-side prep. Returns (shared dict, per-core list of dicts)."""
    n_inst = 2 * n_iter
    bpc = B // N_CORES                      # samples per core (full cfg: 32)
    n_groups = 2 if spg == 16 else 1        # small configs: single group
    use = n_groups * spg                    # samples actually computed per core

    p1 = np.asarray(inputs['p_array1']).astype(np.int64)
    p2 = np.asarray(inputs['p_array2']).astype(np.int64)
    inv1 = np.argsort(p1)
    inv2 = np.argsort(p2)
    t1 = inv2[p1]
    t2 = inv1[p2]

    received = np.asarray(inputs['received'], np.float32)
    r_sys, r_par1, r_par2 = received[:, :, 0], received[:, :, 1], received[:, :, 2]
    s1_sys = r_sys[:, p1]
    s1_par = r_par2[:, inv2[p1]]
    s2_sys = r_sys[:, p2]
    s2_par = r_par1[:, inv1[p2]]

    conv0_w = np.asarray(inputs['conv0_w'], np.float32)
    conv0_b = np.asarray(inputs['conv0_b'], np.float32)
    convs_w = np.asarray(inputs['convs_w'], np.float32)
    convs_b = np.asarray(inputs['convs_b'], np.float32)
    lin1_w = np.asarray(inputs['lin1_w'], np.float32)
    lin1_b = np.asarray(inputs['lin1_b'], np.float32)
    lin2_w = np.asarray(inputs['lin2_w'], np.float32)
    lin2_b = np.asarray(inputs['lin2_b'], np.float32)
    lin2_last_w = np.asarray(inputs['lin2_last_w'], np.float32)

    # masked conv0 lhsT: [inst, 128, 20*100]; row 32j+8v+7 of the (v, k=2)
    # block carries conv0_b + 1 (paired with the constant-1 input row)
    c0m = np.zeros((n_inst, 128, 20 * UNIT), np.float32)
    # mid-layer lhsT: [inst, 101, 20*100]; row 100 of each k=2 block carries
    # b' + 1 = b - sum(W) + 1 (offset-representation bias correction)
    cwm = np.zeros((n_inst, CW_ROWS, 20 * UNIT), np.float32)
    # fused linear lhsT: rows 0..99 = w^T, 100 = bias - sum(w) (const-1 row),
    # 101..105 = -I (extrinsic)
    linw = np.zeros((UNIT + 1 + FT, n_inst, FT), np.float32)
    linlast = np.zeros((UNIT + 1 + FT, 1), np.float32)
    linlast[:UNIT, 0] = lin2_last_w[0]
    linlast[UNIT, 0] = -lin2_last_w[0].sum()

    for inst in range(n_inst):
        idx, col = divmod(inst, 2)
        w0 = conv0_w[idx, col]              # [100, 7, 5]
        b0 = conv0_b[idx, col]
        for v in range(4):
            for k in range(K):
                blk = c0m[inst, :, (v * K + k) * UNIT:(v * K + k + 1) * UNIT]
                for j in range(4):
                    blk[32 * j + 8 * v:32 * j + 8 * v + 7, :] = w0[:, :, k].T
                    if k == 2:
                        blk[32 * j + 8 * v + 7, :] = b0 + 1.0
        for li in range(1, NL):
            wl = convs_w[idx, col, li - 1]  # [100out, 100in, 5]
            bl = convs_b[idx, col, li - 1] - wl.sum(axis=(1, 2))
            for k in range(K):
                blk = cwm[inst, :, ((li - 1) * K + k) * UNIT:
                          ((li - 1) * K + k + 1) * UNIT]
                blk[:UNIT, :] = wl[:, :, k].T
                if k == 2:
                    blk[UNIT, :] = bl + 1.0
        if inst < n_inst - 1:
            lw = lin1_w[idx] if col == 0 else lin2_w[idx]
            lb = lin1_b[idx] if col == 0 else lin2_b[idx]
            linw[:UNIT, inst, :] = lw.T
            linw[UNIT, inst, :] = lb - lw.sum(axis=1)
            if inst > 0:
                linw[UNIT + 1:, inst, :] = -np.eye(FT, dtype=np.float32)

    idx_t1 = _wrap_idx(t1, 8)
    idx_t2 = _wrap_idx(t2, 8)
    idx_o = _wrap_idx(inv2, 1)

    # init pattern for TB: constant-1 conv0 bias rows (incl. halo cols)
    tb_init = np.zeros((128, L + 4), np.float32)
    tb_init[7::8, :] = 1.0
    # init pattern for XB tiles: row 100 = const 1 (bias row), halo cols of
    # activation rows = 1 (offset representation of zero padding)
    xb_init = np.zeros((UNIT + 1 + FT, L + 4), np.float32)
    xb_init[UNIT, :] = 1.0
    xb_init[:UNIT, 0:2] = 1.0
    xb_init[:UNIT, L + 2:L + 4] = 1.0

    shared = {
        'c0m': c0m, 'cwm': cwm,
        'linw': np.ascontiguousarray(linw.reshape(UNIT + 1 + FT, -1)),
        'linlast': linlast,
        'idx_t1': idx_t1, 'idx_t2': idx_t2, 'idx_o': idx_o,
        'tb_init': tb_init, 'xb_init': xb_init,
    }

    per_core = []
    for c in range(N_CORES):
        lo = c * bpc
        ta = np.zeros((n_groups, 128, L + 4), np.float32)
        s1i = np.zeros((n_groups, 128, L), np.float32)
        s2i = np.zeros((n_groups, 128, L), np.float32)
        for g in range(n_groups):
            for si in range(spg):
                s = lo + g * spg + si
                ta[g, 8 * si + 0, 2:L + 2] = s1_sys[s]
                ta[g, 8 * si + 1, 2:L + 2] = r_par1[s]
                ta[g, 8 * si + 7, :] = 1.0        # constant-1 bias row
                s1i[g, 8 * si + 0] = s1_sys[s]
                s1i[g, 8 * si + 1] = s1_par[s]
                s1i[g, 8 * si + 7] = 1.0          # gathers to constant 1
                s2i[g, 8 * si + 0] = s2_sys[s]
                s2i[g, 8 * si + 1] = s2_par[s]
                s2i[g, 8 * si + 7] = 1.0
        m = dict(shared)
        m['ta_init'] = ta
        m['s1_init'] = s1i
        m['s2_init'] = s2i
        per_core.append(m)
    return shared, per_core, use


def build_program(spg=16, n_iter=NUM_ITER):
    """Emit the Bass/Tile program. spg = samples per group (<=16)."""
    n_inst = 2 * n_iter
    n_groups = 2 if spg == 16 else 1
    ch = 8 * spg                       # used octet rows (128 at spg=16)
    chg = ((ch + 15) // 16) * 16       # gather channels (mult of 16)

    nc = bacc.Bacc('TRN2', target_bir_lowering=False, debug=False)

    ta_d = nc.dram_tensor("ta_init", [n_groups, 128, L + 4], F32R,
                          kind="ExternalInput")
    s1_d = nc.dram_tensor("s1_init", [n_groups, 128, L], F32, kind="ExternalInput")
    s2_d = nc.dram_tensor("s2_init", [n_groups, 128, L], F32, kind="ExternalInput")
    c0m_d = nc.dram_tensor("c0m", [n_inst, 128, 20 * UNIT], F32R,
                           kind="ExternalInput")
    cwm_d = nc.dram_tensor("cwm", [n_inst, CW_ROWS, 20 * UNIT], F32R,
                           kind="ExternalInput")
    lw_d = nc.dram_tensor("linw", [UNIT + 1 + FT, n_inst * FT], F32R,
                          kind="ExternalInput")
    ll_d = nc.dram_tensor("linlast", [UNIT + 1 + FT, 1], F32R,
                          kind="ExternalInput")
    tb_d = nc.dram_tensor("tb_init", [128, L + 4], F32R, kind="ExternalInput")
    xbi_d = nc.dram_tensor("xb_init", [UNIT + 1 + FT, L + 4], F32R,
                           kind="ExternalInput")
    it1_d = nc.dram_tensor("idx_t1", [128, L // 16], I16, kind="ExternalInput")
    it2_d = nc.dram_tensor("idx_t2", [128, L // 16], I16, kind="ExternalInput")
    io_d = nc.dram_tensor("idx_o", [16, L // 16], I16, kind="ExternalInput")
    out_d = nc.dram_tensor("out", [n_groups * spg, L], F32, kind="ExternalOutput")

    with tile.TileContext(nc) as tc:
        with tc.tile_pool(name="persist", bufs=1) as pp, \
             tc.tile_pool(name="wts", bufs=2) as wp, \
             tc.tile_pool(name="elu", bufs=2) as ep, \
             tc.tile_pool(name="ps", bufs=4, space="PSUM") as ps:

            # ---- persistent tiles ----
            TA = [pp.tile([128, L + 4], F32R, tag=f"TA{g}", name=f"TA{g}") for g in range(n_groups)]
            TB = [pp.tile([128, L + 4], F32R, tag=f"TB{g}", name=f"TB{g}") for g in range(n_groups)]
            Tf = [pp.tile([128, L], F32, tag=f"Tf{g}", name=f"Tf{g}") for g in range(n_groups)]
            S1 = [pp.tile([128, L], F32, tag=f"S1{g}", name=f"S1{g}") for g in range(n_groups)]
            S2 = [pp.tile([128, L], F32, tag=f"S2{g}", name=f"S2{g}") for g in range(n_groups)]
            XB = [[pp.tile([UNIT + 1 + FT if i == 1 else CW_ROWS, L + 4], F32R,
                           tag=f"XB{p}_{i}", name=f"XB{p}_{i}") for i in range(3)]
                  for p in range(2)]
            negone = pp.tile([UNIT, 1], F32, tag="negone")
            linw_t = pp.tile([UNIT + 1 + FT, n_inst * FT], F32R, tag="linw")
            linlast_t = pp.tile([UNIT + 1 + FT, 1], F32R, tag="linlast")
            it1_t = pp.tile([128, L // 16], I16, tag="it1")
            it2_t = pp.tile([128, L // 16], I16, tag="it2")
            io_t = pp.tile([16, L // 16], I16, tag="io")

            # ---- init ----
            for g in range(n_groups):
                nc.vector.memset(Tf[g][:, :], 0.0)
                nc.sync.dma_start(out=TB[g], in_=tb_d[:, :])
                nc.sync.dma_start(out=TA[g], in_=ta_d[g])
                nc.sync.dma_start(out=S1[g], in_=s1_d[g])
                nc.sync.dma_start(out=S2[g], in_=s2_d[g])
            for pset in XB:
                for xb in pset:
                    rows = xb.shape[0]
                    nc.sync.dma_start(out=xb, in_=xbi_d[0:rows, :])
            nc.vector.memset(negone[:, :], -1.0)
            nc.sync.dma_start(out=linw_t, in_=lw_d[:, :])
            nc.sync.dma_start(out=linlast_t, in_=ll_d[:, :])
            nc.sync.dma_start(out=it1_t, in_=it1_d[:, :])
            nc.sync.dma_start(out=it2_t, in_=it2_d[:, :])
            nc.sync.dma_start(out=io_t, in_=io_d[:, :])

            def elu(psum, xout, xlo):
                """xout[0:100, xlo+2 : xlo+1026] = elu(z)+1, psum half = z+1.

                elu(z)+1 = max(min(exp(z), 1), z+1); exp(z) = exp(psum - 1).
                ACT writes exp(z) straight into xout; the fused combine then
                runs in place (no staging tile, no ring coupling).
                """
                n = L // 2
                sl = xout[0:UNIT, xlo + 2:xlo + 2 + n]
                nc.scalar.activation(sl, psum[:, 0:n], AF.Exp,
                                     bias=negone[:, 0:1])
                nc.vector.scalar_tensor_tensor(sl, sl, 1.0, psum[:, 0:n],
                                               ALU.min, ALU.max)

            for inst in range(n_inst):
                idx, col = divmod(inst, 2)
                last = inst == n_inst - 1
                c0t = wp.tile([128, 20 * UNIT], F32R, tag="c0t")
                cwt = wp.tile([CW_ROWS, 20 * UNIT], F32R, tag="cwt")
                nc.sync.dma_start(out=c0t, in_=c0m_d[inst])
                nc.sync.dma_start(out=cwt, in_=cwm_d[inst])

                # gathers feeding this column's conv0 input
                for g in range(n_groups):
                    if col == 0:
                        if idx == 0:
                            continue        # TA preloaded (prior = 0)
                        nc.gpsimd.ap_gather(Tf[g][0:chg, :], S2[g][0:chg, :],
                                            it1_t[0:chg, :], channels=chg,
                                            num_elems=L, d=1, num_idxs=L)
                        nc.scalar.copy(TA[g][:, 2:L + 2], Tf[g][:, :])
                    else:
                        nc.gpsimd.ap_gather(Tf[g][0:chg, :], S1[g][0:chg, :],
                                            it2_t[0:chg, :], channels=chg,
                                            num_elems=L, d=1, num_idxs=L)
                        nc.scalar.copy(TB[g][:, 2:L + 2], Tf[g][:, :])

                for g in range(n_groups):
                    T = TA[g] if col == 0 else TB[g]

                    def conv_layer(sp, si, li):
                        """One conv layer for sample si (pair slot sp)."""
                        j, v = divmod(si, 4)
                        for h in range(2):
                            pt = ps.tile([UNIT, L // 2], F32, tag="ps",
                                         name="pt")
                            for cc in range(2):
                                c = 2 * h + cc
                                for t, k in enumerate(TAPS):
                                    if li == 0:
                                        nc.tensor.matmul(
                                            pt[:, cc * 512:(cc + 1) * 512],
                                            c0t[32 * j:32 * j + 32,
                                                (v * K + k) * UNIT:
                                                (v * K + k + 1) * UNIT],
                                            T[32 * j:32 * j + 32,
                                              c * 512 + k:c * 512 + k + 512],
                                            start=(t == 0), stop=(t == 4),
                                            tile_position=(32 * j, 0))
                                    else:
                                        xin = XB[sp][(li - 1) % 3]
                                        nc.tensor.matmul(
                                            pt[:, cc * 512:(cc + 1) * 512],
                                            cwt[:, ((li - 1) * K + k) * UNIT:
                                                ((li - 1) * K + k + 1) * UNIT],
                                            xin[0:CW_ROWS,
                                                c * 512 + k:c * 512 + k + 512],
                                            start=(t == 0), stop=(t == 4))
                            elu(pt, XB[sp][li % 3], h * (L // 2))

                    def lin_stage(sp, si):
                        x5 = XB[sp][1]
                        rows = slice(8 * si + 2, 8 * si + 7)
                        if inst > 0:
                            nc.sync.dma_start(
                                out=x5[UNIT + 1:UNIT + 1 + FT, 2:L + 2],
                                in_=Tf[g][rows, :].bitcast(F32R))
                        m = 1 if last else FT
                        wsl = linlast_t[:, 0:1] if last \
                            else linw_t[:, inst * FT:inst * FT + FT]
                        for h in range(2):
                            hl = h * (L // 2)
                            pslin = ps.tile([m, L // 2], F32, tag="ps",
                                            name="pslin")
                            for cc in range(2):
                                c = 2 * h + cc
                                nc.tensor.matmul(
                                    pslin[:, cc * 512:(cc + 1) * 512], wsl,
                                    x5[:, c * 512 + 2:c * 512 + 514],
                                    start=True, stop=True)
                            if last:
                                stg1 = ep.tile([1, L // 2], F32, tag="stg1",
                                               name="stg1")
                                nc.scalar.activation(stg1, pslin[0:1, :],
                                                     AF.Tanh, scale=0.5)
                                nc.vector.tensor_scalar(stg1, stg1, 0.5, 0.5,
                                                        ALU.mult, ALU.add)
                                nc.sync.dma_start(
                                    out=Tf[g][si:si + 1, hl:hl + L // 2],
                                    in_=stg1)
                            else:
                                Sd = S1[g] if col == 0 else S2[g]
                                stg = ep.tile([FT, L // 2], F32, tag="stg",
                                              name="stg")
                                nc.scalar.copy(stg, pslin)
                                nc.sync.dma_start(out=Sd[rows, hl:hl + L // 2],
                                                  in_=stg)

                    for p in range(spg // 2):
                        sA, sB = 2 * p, 2 * p + 1
                        for li in range(NL):
                            conv_layer(0, sA, li)
                            conv_layer(1, sB, li)
                        lin_stage(0, sA)
                        lin_stage(1, sB)

            # final: out[l] = sig[inv2[l]]
            for g in range(n_groups):
                nc.gpsimd.ap_gather(S1[g][0:16, :], Tf[g][0:16, :],
                                    io_t[0:16, :], channels=16,
                                    num_elems=L, d=1, num_idxs=L)
                nc.sync.dma_start(out=out_d[g * spg:g * spg + spg, :],
                                  in_=S1[g][0:spg, :])

    nc.compile()
    return nc


_PROG_CACHE = {}


def _get_prog(spg, n_iter):
    key = (spg, n_iter)
    if key not in _PROG_CACHE:
        _PROG_CACHE[key] = build_program(spg, n_iter)
    return _PROG_CACHE[key]


def run(inputs, spg=16, n_iter=NUM_ITER, cores=N_CORES, **spmd_kwargs):
    nc = _get_prog(spg, n_iter)
    _, per_core, use = build_host_inputs(inputs, spg, n_iter)
    res = run_bass_kernel_spmd(nc, per_core[:cores],
                               core_ids=list(range(cores)), **spmd_kwargs)
    return res, use


def kernel(**inputs):
    res, use = run(inputs)
    bpc = B // N_CORES
    out = np.empty((B, L, 1), np.float32)
    for c in range(N_CORES):
        out[c * bpc:(c + 1) * bpc, :, 0] = res.results[c]["out"]
    return out
